# revision 3
# baseline (speedup 1.0000x reference)
"""Trainium2 Bass kernel for nn_DiscriminativeLoss_86242943304305.

The reference loss is einsum('bfl,blk->', pred, one_hot(target)) with
target values always in [0, 16) == the one-hot bin count, so the mask
term sums to exactly 1.0 at every pixel and the loss equals
prediction.sum().  The kernel is therefore a pure memory-bound global
sum of the [16, 8, 512, 512] f32 prediction tensor; `target` never
needs to be read.

Sharding: data-parallel over the batch axis — core i reduces batches
[2i, 2i+2) (16 MiB each) to [128, 1] partials; the host sums the 8*128
partials (the "all-reduce" of the sharding hint, done host-side since
the output is a single scalar).
"""

import numpy as np

_N_CORES = 8
_B, _F, _H, _W = 16, 8, 512, 512
_ELEMS_PER_CORE = (_B // _N_CORES) * _F * _H * _W  # 4,194,304
_P = 128
_TILE_M = 4096
_NTILES = _ELEMS_PER_CORE // (_P * _TILE_M)  # 8
_BUFS = 4

_cached_nc = None


def _build():
    global _cached_nc
    if _cached_nc is not None:
        return _cached_nc

    import concourse.bacc as bacc
    import concourse.mybir as mybir
    from concourse.tile import TileContext

    nc = bacc.Bacc(
        "TRN2", target_bir_lowering=False, debug=False, num_devices=_N_CORES
    )
    x = nc.dram_tensor(
        "x", [_NTILES, _P, _TILE_M], mybir.dt.float32, kind="ExternalInput"
    )
    out = nc.dram_tensor("out", [_P, 1], mybir.dt.float32, kind="ExternalOutput")

    with TileContext(nc) as tc:
        with (
            tc.tile_pool(name="stream", bufs=_BUFS) as pool,
            tc.tile_pool(name="acc", bufs=1) as accp,
        ):
            acc = accp.tile([_P, _NTILES], mybir.dt.float32)
            for i in range(_NTILES):
                t = pool.tile([_P, _TILE_M], mybir.dt.float32, tag="stream")
                nc.sync.dma_start(t[:], x[i])
                nc.vector.reduce_sum(acc[:, i : i + 1], t[:], axis=mybir.AxisListType.X)
            total = accp.tile([_P, 1], mybir.dt.float32, tag="tot")
            nc.vector.reduce_sum(total[:], acc[:], axis=mybir.AxisListType.X)
            nc.sync.dma_start(out[:], total[:])

    nc.compile()
    _cached_nc = nc
    return nc


def kernel(prediction: np.ndarray, target: np.ndarray) -> np.ndarray:
    from concourse.bass_utils import run_bass_kernel_spmd

    pred = np.ascontiguousarray(prediction, dtype=np.float32).reshape(
        _N_CORES, _NTILES, _P, _TILE_M
    )
    in_maps = [{"x": pred[i]} for i in range(_N_CORES)]
    nc = _build()
    res = run_bass_kernel_spmd(nc, in_maps, core_ids=list(range(_N_CORES)))
    partials = np.stack([r["out"] for r in res.results])
    total = partials.astype(np.float64).sum()
    return np.array(total, dtype=np.float32)
